# revision 79
# baseline (speedup 1.0000x reference)
"""Trainium2 Bass kernel for a full MHA block (QKV proj + softmax attention +
output proj + residual + LayerNorm), B=2, S=4096, E=512, H=8, D=64.

Sharding: sequence-parallel over 8 cores (4 seq shards x 2 batches). Each core
owns R=1024 query rows of one batch, recomputes K/V for the full context
(avoids all cross-core communication), and writes its own [R, E] output slice.

Layout strategy (per core):
  - x^T via XBAR DMA-transpose (bf16) -> [128, 4, S]
  - K^T, Q^T projections in head-major layout [e_out(=2 heads)/128, t]
  - scores computed transposed: S_T[t, s] = K^T.T @ Q^T, two heads packed into
    PE row groups (K=64 each), PSUM out [t=128, s=512]
  - exp on ScalarE straight from PSUM (scale=1/8 free), out bf16 A_T tiles
  - A@V: lhsT = [V_h | ones] (65 cols) so row 64 of the product accumulates the
    softmax denominator for free; accumulated per 3-chunk group in PSUM then
    drained into an SBUF f32 accumulator by VectorE (keeps PSUM pressure at 8
    banks: 2x3 score staging + 2 utility)
  - normalize: reciprocal of denom row, partition-broadcast via SBUF->SBUF DMA,
    VectorE multiply -> ctx^T bf16 (per-head tiles, base partition 0 always)
  - O-proj: per-head K=64 matmuls accumulating all 8 heads, + residual (f32) +
    LayerNorm (bn_stats/bn_aggr) on VectorE/ScalarE, f32 output.
"""

import sys

sys.path.insert(0, "/opt/trn_rl_repo")

import numpy as np
import ml_dtypes

import concourse.bass as bass
import concourse.bacc as bacc
import concourse.mybir as mybir
import concourse.tile as tile
from concourse.bass import ds, ts

# Problem constants (hardcoded per harness contract)
B = 2
S = 4096
E = 512
H = 8
D = 64
N_CORES = 8
SEQ_SHARDS = N_CORES // B
R = S // SEQ_SHARDS  # 1024 own query rows per core

F32 = mybir.dt.float32
F32R = mybir.dt.float32r
F16 = mybir.dt.float16
FP8 = mybir.dt.float8e4
VP = 80  # padded V columns (64 V + 1 ones + pad to a 16-multiple for DoubleRow)
EXP_SHIFT = -3.0  # exp(s/8 - 3): keeps exp outputs < fp8e4 max; cancels in softmax
# log-domain exp on DVE: fp8e4m3 bits of exp(s/8+SHIFT) == s*BITS_MUL + BITS_ADD,
# rounded + saturated to [0,255] by the uint8 convert (verified on HW)
BITS_MUL = 11.5416529 / 8.0
BITS_ADD = 56.0 + 11.5416529 * EXP_SHIFT
EXP_DVE_MOD = 10  # 2 of every 10 exp tiles go to DVE (bit-trick), rest to Scalar
AF = mybir.ActivationFunctionType


def build_mha(nc, seq=S, rows=R, exp_group=2, sblk=512, exp_dve_mod=EXP_DVE_MOD):
    """Emit the Tile program. seq/rows shrinkable for simulation."""
    P = 128
    EC = E // P           # 4 e_in chunks
    HPAIRS = H // 2       # 4 head-pair blocks (=e_out blocks of 128)
    TT = seq // P         # t tiles
    kblk = min(512, seq)
    TB = seq // kblk      # t blocks for K-proj
    qblk = min(512, rows)
    QB = rows // qblk     # r blocks for Q-proj
    sblk = min(sblk, rows)
    SB = rows // sblk     # s blocks per core
    ST = rows // P        # s tiles for O-proj/LN

    G2 = E // 256         # DoubleRow chunk-pair groups for the projections

    # ---- DRAM I/O ----
    # x fed pre-transposed+packed fp8 from host:
    #   xT[p, g, i, t] = x[t, (2g+i)*128+p]   (DoubleRow pair axis i)
    xT_d = nc.dram_tensor("xT_f8", [P, G2, 2, seq], FP8, kind="ExternalInput").ap()
    xoT_d = nc.dram_tensor("xoT_f8", [P, G2, 2, rows], FP8, kind="ExternalInput").ap()
    xo_f32 = nc.dram_tensor("xo_f32", [rows, E], F32, kind="ExternalInput").ap()
    # weights pre-packed on host: wX[p, g, i, e] = w[(2g+i)*128+p, e];
    # wo[p, h, e] = wo[h*64+p, e]
    wq = nc.dram_tensor("wq_f8", [P, G2, 2, E], FP8, kind="ExternalInput").ap()
    wk = nc.dram_tensor("wk_f8", [P, G2, 2, E], FP8, kind="ExternalInput").ap()
    wv = nc.dram_tensor("wv_f8", [P, G2, 2, E], FP8, kind="ExternalInput").ap()
    wo = nc.dram_tensor("wo_f16", [D, H, E], F16, kind="ExternalInput").ap()
    bq = nc.dram_tensor("bq", [E], F32, kind="ExternalInput").ap()
    bk = nc.dram_tensor("bk", [E], F32, kind="ExternalInput").ap()
    bv = nc.dram_tensor("bv", [E], F32, kind="ExternalInput").ap()
    bo = nc.dram_tensor("bo", [E], F32, kind="ExternalInput").ap()
    ln_g = nc.dram_tensor("ln_g", [E], F32, kind="ExternalInput").ap()
    ln_b = nc.dram_tensor("ln_b", [E], F32, kind="ExternalInput").ap()
    y_out = nc.dram_tensor("y", [rows, E], F32, kind="ExternalOutput").ap()

    with tile.TileContext(nc) as tc:
        with (
            tc.tile_pool(name="singles", bufs=1) as singles,
            tc.tile_pool(name="kqv", bufs=1) as kqv,
            tc.tile_pool(name="vtiles", bufs=TT) as vtiles,
            tc.tile_pool(name="at", bufs=4) as atp,
            tc.tile_pool(name="ctx", bufs=4) as ctxp,
            tc.tile_pool(name="norm", bufs=4) as normp,
            tc.tile_pool(name="yout", bufs=3) as youtp,
            tc.tile_pool(name="stg", bufs=2, space="PSUM") as stg,
            tc.tile_pool(name="acc", bufs=2, space="PSUM") as accp,
            tc.tile_pool(name="util", bufs=2, space="PSUM") as util,
        ):
            # ---------- constants / weights ----------
            # startup-critical order: wk[hp0] -> xT(tb0) -> wq[hp0] -> xoT ->
            # wv -> rest (interleaved below with the xT loads)
            wq_sb = singles.tile([P, G2, 2, E], FP8, name="wq_sb")
            wk_sb = singles.tile([P, G2, 2, E], FP8, name="wk_sb")
            wv_sb = singles.tile([P, G2, 2, E], FP8, name="wv_sb")
            nc.sync.dma_start(wk_sb[:, :, :, 0:P], wk[:, :, :, 0:P])
            # wo in per-head layout: [64, H, E]
            wo_sb = singles.tile([D, H, E], F16, name="wo_sb")
            # per-e_out-column biases for k^T/q^T ([128, 4] with col = block)
            bk_sb = singles.tile([P, EC], F32, name="bk_sb")
            bq_sb = singles.tile([P, EC], F32, name="bq_sb")
            nc.gpsimd.dma_start(bk_sb, bk.rearrange("(c p) -> p c", p=P))
            nc.gpsimd.dma_start(bq_sb, bq.rearrange("(c p) -> p c", p=P))
            # free-dim broadcast tiles
            bv_bc = singles.tile([P, E], F32, name="bv_bc")
            bo_bc = singles.tile([P, E], F32, name="bo_bc")
            g_bc = singles.tile([P, E], F32, name="g_bc")
            b_bc = singles.tile([P, E], F32, name="b_bc")
            for dst, src in ((bv_bc, bv), (bo_bc, bo), (g_bc, ln_g), (b_bc, ln_b)):
                nc.gpsimd.dma_start(out=dst, in_=src[None, :].to_broadcast((P, E)))
            eps_t = singles.tile([P, 1], F32, name="eps_t")
            nc.vector.memset(eps_t, 1e-5)
            shift_t = singles.tile([P, 1], F32, name="shift_t")
            nc.vector.memset(shift_t, EXP_SHIFT)
            ones16 = singles.tile([1, D], F16, name="ones16")
            nc.vector.memset(ones16, 1.0)
            # for folding bo into the O-proj as a K=1 matmul
            ones_p = singles.tile([1, P], F16, name="ones_p")
            nc.vector.memset(ones_p, 1.0)
            bo16 = singles.tile([1, E], F16, name="bo16")
            nc.scalar.activation(out=bo16, in_=bo_bc[0:1, :], func=AF.Copy)

            # ---------- x^T loads (pre-transposed fp8 on host) ----------
            xT = singles.tile([P, G2, 2, seq], FP8, name="xT")
            xoT = singles.tile([P, G2, 2, rows], FP8, name="xoT")
            def _w_rest_0():
                nc.gpsimd.dma_start(wq_sb[:, :, :, 0:P], wq[:, :, :, 0:P])
                for g in range(G2):
                    eng = nc.sync if g % 2 == 0 else nc.gpsimd
                    eng.dma_start(xoT[:, g, :, :], xoT_d[:, g, :, :])
                nc.sync.dma_start(wv_sb, wv)

            def _w_rest_1():
                nc.gpsimd.dma_start(wk_sb[:, :, :, P:E], wk[:, :, :, P:E])
                nc.sync.dma_start(wq_sb[:, :, :, P:E], wq[:, :, :, P:E])

            def _w_rest_2():
                nc.gpsimd.dma_start(wo_sb, wo)

            w_rest = [_w_rest_0, _w_rest_1, _w_rest_2]
            for tb in range(TB):
                for g in range(G2):
                    eng = nc.sync if g % 2 == 0 else nc.gpsimd
                    eng.dma_start(
                        xT[:, g, :, ds(tb * kblk, kblk)],
                        xT_d[:, g, :, ds(tb * kblk, kblk)],
                    )
                if w_rest:
                    w_rest.pop(0)()
            while w_rest:
                w_rest.pop(0)()

            # ---------- V projection (+bias, +ones col) ----------
            # fp8 DoubleRow pair layout: v2[pair][p, i, h, 0:64] = v[t=2*pair+i],
            # [..., 64] = 1.0 (softmax denominator column), rest zero-padding.
            v_tiles = {}
            def emit_v(t):
                pair, i = divmod(t, 2)
                if i == 0:
                    vt = vtiles.tile([P, 2, H, VP], FP8, name=f"v_{pair}", tag="v")
                    nc.vector.memset(vt[:, :, :, D:VP], 0.0)
                    nc.vector.memset(vt[:, :, :, D : D + 1], 1.0)
                    v_tiles[pair] = vt
                vt = v_tiles[pair]
                ps = util.tile([P, E], F32, name="v_ps", tag="u")
                for g in range(G2):
                    nc.tensor.matmul(
                        ps, lhsT=xT[:, g, :, ts(t, P)], rhs=wv_sb[:, g, :, :],
                        start=(g == 0), stop=(g == G2 - 1),
                        perf_mode=mybir.MatmulPerfMode.DoubleRow,
                    )
                nc.vector.tensor_add(
                    out=vt[:, i, :, 0:D],
                    in0=ps.rearrange("p (h d) -> p h d", h=H),
                    in1=bv_bc.rearrange("p (h d) -> p h d", h=H),
                )

            # ---------- K^T / Q^T projections (per head-pair block) ----------
            kT = [kqv.tile([P, seq], F16, name=f"kT_{hp}") for hp in range(HPAIRS)]
            qT = [kqv.tile([P, rows], F16, name=f"qT_{hp}") for hp in range(HPAIRS)]

            def emit_k(hp, tb):
                ps = util.tile([P, 512], F32, name="k_ps", tag="u")
                for g in range(G2):
                    nc.tensor.matmul(
                        ps[:, :kblk], lhsT=wk_sb[:, g, :, ds(hp * P, P)],
                        rhs=xT[:, g, :, ds(tb * kblk, kblk)],
                        start=(g == 0), stop=(g == G2 - 1),
                        perf_mode=mybir.MatmulPerfMode.DoubleRow,
                    )
                nc.vector.tensor_tensor(
                    kT[hp][:, ds(tb * kblk, kblk)], ps[:, :kblk],
                    bk_sb[:, hp : hp + 1].to_broadcast((P, kblk)),
                    mybir.AluOpType.add,
                )

            def emit_q(hp, rb):
                ps = util.tile([P, 512], F32, name="q_ps", tag="u")
                for g in range(G2):
                    nc.tensor.matmul(
                        ps[:, :qblk], lhsT=wq_sb[:, g, :, ds(hp * P, P)],
                        rhs=xoT[:, g, :, ds(rb * qblk, qblk)],
                        start=(g == 0), stop=(g == G2 - 1),
                        perf_mode=mybir.MatmulPerfMode.DoubleRow,
                    )
                nc.vector.tensor_tensor(
                    qT[hp][:, ds(rb * qblk, qblk)], ps[:, :qblk],
                    bq_sb[:, hp : hp + 1].to_broadcast((P, qblk)),
                    mybir.AluOpType.add,
                )

            # ---------- attention ----------
            scale = 1.0 / np.sqrt(D)
            exp_ctr = [0]
            # deferred normalize closures: each block's [recip -> PE
            # broadcast -> ctxT multiply] tail is emitted a few groups into
            # the NEXT block's attention, so the in-order PE queue never
            # stalls on the drain/reciprocal chain at block boundaries.
            pending_norm = []
            # ctx^T accumulators (f32, SBUF), one per head in the pair
            def attention(hp, sb, fillers, dve_exp=True, norm_first=False):
                if norm_first and pending_norm:
                    pending_norm.pop(0)()
                ctx_ps = [
                    accp.tile([VP, sblk], F32, name=f"ctx_{h}", tag="ctx")
                    for h in range(2)
                ]
                n_groups = (TT + exp_group - 1) // exp_group
                pending = []  # (at_tile, t0, gsz) awaiting A@V, one group behind
                def flush_av(last):
                    at_p, t0p, gszp = pending.pop(0)
                    assert gszp == 2, "DoubleRow A@V needs t-tile pairs"
                    for h in range(2):
                        nc.tensor.matmul(
                            ctx_ps[h][:, :sblk],
                            lhsT=v_tiles[t0p // 2][:, :, hp * 2 + h, :],
                            rhs=at_p[h][:, :, :sblk],
                            start=(t0p == 0), stop=last,
                            perf_mode=mybir.MatmulPerfMode.DoubleRow,
                        )
                for g in range(n_groups):
                    if g == 2 and not norm_first and pending_norm:
                        pending_norm.pop(0)()
                    t0 = g * exp_group
                    gsz = min(exp_group, TT - t0)
                    at_pair = []
                    for h in range(2):
                        st_t = stg.tile([P, exp_group, 512], F32, name=f"stg_{h}", tag="stg")
                        # scores (2 heads packed via PE row groups)
                        for j in range(gsz):
                            nc.tensor.matmul(
                                st_t[:, j, :sblk],
                                lhsT=kT[hp][ds(h * D, D), ts(t0 + j, P)],
                                rhs=qT[hp][ds(h * D, D), ds(sb * sblk, sblk)],
                                start=True, stop=True,
                                tile_position=(h * D, 0),
                            )
                        # exp (with 1/sqrt(D) folded in), PSUM -> SBUF fp8.
                        # Split across Scalar (true exp) and DVE (log-domain
                        # bit trick) to break the Scalar-engine bottleneck.
                        at_t = atp.tile([P, exp_group, 512], FP8, name=f"at_{h}", tag="at")
                        if (dve_exp and exp_dve_mod > 0
                                and (exp_ctr[0] * 2) % exp_dve_mod < 2):
                            nc.vector.tensor_scalar(
                                out=at_t[:, :gsz, :sblk].bitcast(mybir.dt.uint8),
                                in0=st_t[:, :gsz, :sblk],
                                scalar1=BITS_MUL, scalar2=BITS_ADD,
                                op0=mybir.AluOpType.mult,
                                op1=mybir.AluOpType.add,
                            )
                        else:
                            nc.scalar.activation(
                                out=at_t[:, :gsz, :sblk], in_=st_t[:, :gsz, :sblk],
                                func=AF.Exp, scale=scale, bias=shift_t,
                            )
                        exp_ctr[0] += 1
                        at_pair.append(at_t)
                    pending.append((at_pair, t0, gsz))
                    if len(pending) > 1:
                        flush_av(False)
                    if fillers:
                        fillers.pop(0)()
                flush_av(True)
                # drain on Scalar (frees DVE + PSUM acc banks); the denom row
                # hops to partition 0 by DMA for the custom-DVE reciprocal
                ctx_sb = [ctxp.tile([D + 1, sblk], F32, name=f"cs_{h}", tag="cs")
                          for h in range(2)]
                den0 = [normp.tile([1, sblk], F32, name=f"den0_{h}", tag="dn")
                        for h in range(2)]
                for h in range(2):
                    nc.scalar.activation(
                        out=ctx_sb[h], in_=ctx_ps[h][: D + 1, :sblk],
                        func=AF.Copy,
                    )
                    nc.sync.dma_start(den0[h], ctx_sb[h][D : D + 1, :])

                def do_norm(hp=hp, sb=sb, ctx_sb=ctx_sb, den0=den0):
                    for h in range(2):
                        recip = normp.tile([1, sblk], F32, name="recip")
                        nc.vector.reciprocal_approx_fast(
                            out=recip, in_=den0[h]
                        )
                        recip16 = normp.tile([1, sblk], F16, name="recip16")
                        nc.scalar.activation(
                            out=recip16, in_=recip, func=AF.Copy
                        )
                        rb_ps = util.tile([D, 512], F32, name="rb_ps", tag="u")
                        nc.tensor.matmul(
                            rb_ps[:, :sblk], lhsT=ones16, rhs=recip16,
                            start=True, stop=True,
                        )
                        nc.vector.tensor_mul(
                            out=ctxT[hp * 2 + h][:, ds(sb * sblk, sblk)],
                            in0=ctx_sb[h][0:D, :], in1=rb_ps[:, :sblk],
                        )

                pending_norm.append(do_norm)

            ctxT = [kqv.tile([D, rows], F16, name=f"ctxT_{h}") for h in range(H)]

            # ---------- O-projection + residual + LayerNorm ----------
            def emit_out(st):
                ps = util.tile([P, E], F32, name="o_ps", tag="u")
                for h in range(H):
                    nc.tensor.matmul(
                        ps, lhsT=ctxT[h][:, ts(st, P)], rhs=wo_sb[:, h, :],
                        start=(h == 0), stop=False,
                    )
                # bo folded in as a K=1 ones x bo outer product
                nc.tensor.matmul(
                    ps, lhsT=ones_p, rhs=bo16, start=False, stop=True
                )
                xo_t = youtp.tile([P, E], F32, name="xo_t")
                nc.sync.dma_start(xo_t, xo_f32[ts(st, P), :])
                y_t = youtp.tile([P, E], F32, name="y_t")
                nc.vector.tensor_add(out=y_t, in0=ps, in1=xo_t)
                # LayerNorm
                stats = normp.tile([P, 6], F32, name="stats")
                nc.vector.bn_stats(out=stats, in_=y_t)
                mv = normp.tile([P, 2], F32, name="mv")
                nc.vector.bn_aggr(out=mv, in_=stats)
                std = normp.tile([P, 1], F32, name="std")
                nc.scalar.activation(
                    out=std, in_=mv[:, 1:2], func=AF.Sqrt, bias=eps_t
                )
                nc.vector.reciprocal(out=std, in_=std)
                # (y - mu) * rstd fused in one pass (per-partition scalars)
                nc.vector.tensor_scalar(
                    out=y_t, in0=y_t, scalar1=mv[:, 0:1], scalar2=std,
                    op0=mybir.AluOpType.subtract, op1=mybir.AluOpType.mult,
                )
                nc.vector.tensor_mul(out=y_t, in0=y_t, in1=g_bc)
                nc.vector.tensor_add(out=y_t, in0=y_t, in1=b_bc)
                nc.sync.dma_start(y_out[ts(st, P), :], y_t)


            # ---------- emission order ----------
            # K(hp0)/Q(hp0) first so attention starts ASAP; V pairs 0-3 up
            # front, the rest streamed as fillers inside hp0's attention
            # (one pair per group, staying >=4 pairs ahead of the A@V flush).
            emitted_out = set()
            tpb = kblk // P  # t-tiles per k-block
            for tb in range(TB):
                emit_k(0, tb)
                if tb == 0:
                    for rb in range(QB):
                        emit_q(0, rb)
                for t in range(tb * tpb, (tb + 1) * tpb):
                    emit_v(t)

            for hp in range(HPAIRS):
                fillers = []
                if hp + 1 < HPAIRS:
                    nhp = hp + 1
                    for tb in range(TB):
                        fillers.append(lambda nhp=nhp, tb=tb: emit_k(nhp, tb))
                    for rb in range(QB):
                        fillers.append(lambda nhp=nhp, rb=rb: emit_q(nhp, rb))
                elif SB > 1:
                    # last head-pair: stream first s-block's output tiles
                    def of(st):
                        def run():
                            emit_out(st)
                            emitted_out.add(st)
                        return run
                per_sb = (len(fillers) + 1) // SB if fillers else 0
                for sb in range(SB):
                    last_att = hp == HPAIRS - 1 and sb == SB - 1 and SB > 1
                    if last_att:
                        chunk = [of(st) for st in range(ST // SB)]
                    else:
                        chunk = fillers[:per_sb]
                        del fillers[:per_sb]
                    # the streamed emit_outs of the last block read ctxT of
                    # the previous block -> its normalize must run up front
                    attention(hp, sb, chunk, norm_first=last_att)
                    for f in chunk:
                        f()
                for f in fillers:
                    f()

            while pending_norm:
                pending_norm.pop(0)()
            for st in range(ST):
                if st not in emitted_out:
                    emit_out(st)

    return nc


_CACHED = {}


def _get_nc(seq=S, rows=R, exp_group=2, sblk=512, exp_dve_mod=EXP_DVE_MOD):
    key = (seq, rows, exp_group, sblk, exp_dve_mod)
    if key not in _CACHED:
        nc = bacc.Bacc("TRN2", target_bir_lowering=False, debug=False,
                       num_devices=N_CORES)
        build_mha(nc, seq=seq, rows=rows, exp_group=exp_group, sblk=sblk,
                  exp_dve_mod=exp_dve_mod)
        nc.compile()
        _CACHED[key] = nc
    return _CACHED[key]


def pack_fp8_dr(x2d):
    """[S, E] f32 -> [128, G2, 2, S] fp8 with out[p, g, i, t] = x[t, (2g+i)*128+p]."""
    f8 = ml_dtypes.float8_e4m3
    s, e = x2d.shape
    return np.ascontiguousarray(
        x2d.T.reshape(e // 256, 2, 128, s).transpose(2, 0, 1, 3).astype(f8)
    )


def packw_fp8_dr(w):
    """[E, E] f32 -> [128, G2, 2, E] fp8 with out[p, g, i, e] = w[(2g+i)*128+p, e]."""
    f8 = ml_dtypes.float8_e4m3
    e_in, e_out = w.shape
    return np.ascontiguousarray(
        np.asarray(w, np.float32)
        .reshape(e_in // 256, 2, 128, e_out)
        .transpose(2, 0, 1, 3)
        .astype(f8)
    )


def packw_o(w):
    """[E, E] f32 -> [64, H, E] f16 with out[p, h, e] = w[h*64+p, e]."""
    w = np.asarray(w, np.float16)
    return np.ascontiguousarray(w.reshape(H, 64, E).transpose(1, 0, 2))


def make_in_maps(inputs):
    """Shard full inputs into per-core input dicts."""
    x = np.asarray(inputs["x"], np.float32)
    shared = {
        "wq_f8": packw_fp8_dr(inputs["wq"]),
        "wk_f8": packw_fp8_dr(inputs["wk"]),
        "wv_f8": packw_fp8_dr(inputs["wv"]),
        "wo_f16": packw_o(inputs["wo"]),
        "bq": np.asarray(inputs["bq"], np.float32),
        "bk": np.asarray(inputs["bk"], np.float32),
        "bv": np.asarray(inputs["bv"], np.float32),
        "bo": np.asarray(inputs["bo"], np.float32),
        "ln_g": np.asarray(inputs["ln_g"], np.float32),
        "ln_b": np.asarray(inputs["ln_b"], np.float32),
    }
    xT_all = [pack_fp8_dr(x[b]) for b in range(B)]
    in_maps = []
    for c in range(N_CORES):
        b, shard = divmod(c, SEQ_SHARDS)
        r0 = shard * R
        m = dict(shared)
        m["xT_f8"] = xT_all[b]
        m["xoT_f8"] = np.ascontiguousarray(xT_all[b][:, :, :, r0 : r0 + R])
        m["xo_f32"] = np.ascontiguousarray(x[b, r0 : r0 + R])
        in_maps.append(m)
    return in_maps


def kernel(**inputs):
    from concourse import bass_utils

    nc = _get_nc()
    in_maps = make_in_maps(inputs)
    res = bass_utils.run_bass_kernel_spmd(nc, in_maps, core_ids=list(range(N_CORES)))
    out = np.empty((B, S, E), np.float32)
    for c in range(N_CORES):
        b, shard = divmod(c, SEQ_SHARDS)
        out[b, shard * R : (shard + 1) * R] = res.results[c]["y"]
    return out



# revision 80
# speedup vs baseline: 1.1352x; 1.1352x over previous
"""Trainium2 Bass kernel for a full MHA block (QKV proj + softmax attention +
output proj + residual + LayerNorm), B=2, S=4096, E=512, H=8, D=64.

Sharding: sequence-parallel over 8 cores (4 seq shards x 2 batches). Each core
owns R=1024 query rows of one batch, recomputes K/V for the full context
(avoids all cross-core communication), and writes its own [R, E] output slice.

Layout strategy (per core):
  - x^T via XBAR DMA-transpose (bf16) -> [128, 4, S]
  - K^T, Q^T projections in head-major layout [e_out(=2 heads)/128, t]
  - scores computed transposed: S_T[t, s] = K^T.T @ Q^T, two heads packed into
    PE row groups (K=64 each), PSUM out [t=128, s=512]
  - exp on ScalarE straight from PSUM (scale=1/8 free), out bf16 A_T tiles
  - A@V: lhsT = [V_h | ones] (65 cols) so row 64 of the product accumulates the
    softmax denominator for free; accumulated per 3-chunk group in PSUM then
    drained into an SBUF f32 accumulator by VectorE (keeps PSUM pressure at 8
    banks: 2x3 score staging + 2 utility)
  - normalize: reciprocal of denom row, partition-broadcast via SBUF->SBUF DMA,
    VectorE multiply -> ctx^T bf16 (per-head tiles, base partition 0 always)
  - O-proj: per-head K=64 matmuls accumulating all 8 heads, + residual (f32) +
    LayerNorm (bn_stats/bn_aggr) on VectorE/ScalarE, f32 output.
"""

import sys

sys.path.insert(0, "/opt/trn_rl_repo")

import numpy as np
import ml_dtypes

import concourse.bass as bass
import concourse.bacc as bacc
import concourse.mybir as mybir
import concourse.tile as tile
from concourse.bass import ds, ts

# Problem constants (hardcoded per harness contract)
B = 2
S = 4096
E = 512
H = 8
D = 64
N_CORES = 8
SEQ_SHARDS = N_CORES // B
R = S // SEQ_SHARDS  # 1024 own query rows per core

F32 = mybir.dt.float32
F32R = mybir.dt.float32r
F16 = mybir.dt.float16
FP8 = mybir.dt.float8e4
VP = 80  # padded V columns (64 V + 1 ones + pad to a 16-multiple for DoubleRow)
EXP_SHIFT = -3.0  # exp(s/8 - 3): keeps exp outputs < fp8e4 max; cancels in softmax
# log-domain exp on DVE: fp8e4m3 bits of exp(s/8+SHIFT) == s*BITS_MUL + BITS_ADD,
# rounded + saturated to [0,255] by the uint8 convert (verified on HW)
BITS_MUL = 11.5416529 / 8.0
BITS_ADD = 56.0 + 11.5416529 * EXP_SHIFT
EXP_DVE_MOD = 8  # 2 of every 8 exp tiles go to DVE (bit-trick), rest to Scalar
AF = mybir.ActivationFunctionType


def build_mha(nc, seq=S, rows=R, exp_group=2, sblk=512, exp_dve_mod=EXP_DVE_MOD):
    """Emit the Tile program. seq/rows shrinkable for simulation."""
    P = 128
    EC = E // P           # 4 e_in chunks
    HPAIRS = H // 2       # 4 head-pair blocks (=e_out blocks of 128)
    TT = seq // P         # t tiles
    kblk = min(512, seq)
    TB = seq // kblk      # t blocks for K-proj
    qblk = min(512, rows)
    QB = rows // qblk     # r blocks for Q-proj
    sblk = min(sblk, rows)
    SB = rows // sblk     # s blocks per core
    ST = rows // P        # s tiles for O-proj/LN

    G2 = E // 256         # DoubleRow chunk-pair groups for the projections

    # ---- DRAM I/O ----
    # x fed pre-transposed+packed fp8 from host:
    #   xT[p, g, i, t] = x[t, (2g+i)*128+p]   (DoubleRow pair axis i)
    xT_d = nc.dram_tensor("xT_f8", [P, G2, 2, seq], FP8, kind="ExternalInput").ap()
    xoT_d = nc.dram_tensor("xoT_f8", [P, G2, 2, rows], FP8, kind="ExternalInput").ap()
    xo_f32 = nc.dram_tensor("xo_f32", [rows, E], F32, kind="ExternalInput").ap()
    # weights pre-packed on host: wX[p, g, i, e] = w[(2g+i)*128+p, e];
    # wo[p, h, e] = wo[h*64+p, e]
    wq = nc.dram_tensor("wq_f8", [P, G2, 2, E], FP8, kind="ExternalInput").ap()
    wk = nc.dram_tensor("wk_f8", [P, G2, 2, E], FP8, kind="ExternalInput").ap()
    wv = nc.dram_tensor("wv_f8", [P, G2, 2, E], FP8, kind="ExternalInput").ap()
    wo = nc.dram_tensor("wo_f16", [D, H, E], F16, kind="ExternalInput").ap()
    bq = nc.dram_tensor("bq", [E], F32, kind="ExternalInput").ap()
    bk = nc.dram_tensor("bk", [E], F32, kind="ExternalInput").ap()
    bv = nc.dram_tensor("bv", [E], F32, kind="ExternalInput").ap()
    bo = nc.dram_tensor("bo", [E], F32, kind="ExternalInput").ap()
    ln_g = nc.dram_tensor("ln_g", [E], F32, kind="ExternalInput").ap()
    ln_b = nc.dram_tensor("ln_b", [E], F32, kind="ExternalInput").ap()
    y_out = nc.dram_tensor("y", [rows, E], F32, kind="ExternalOutput").ap()

    with tile.TileContext(nc) as tc:
        with (
            tc.tile_pool(name="singles", bufs=1) as singles,
            tc.tile_pool(name="kqv", bufs=1) as kqv,
            tc.tile_pool(name="vtiles", bufs=TT) as vtiles,
            tc.tile_pool(name="at", bufs=4) as atp,
            tc.tile_pool(name="ctx", bufs=4) as ctxp,
            tc.tile_pool(name="norm", bufs=4) as normp,
            tc.tile_pool(name="yout", bufs=3) as youtp,
            tc.tile_pool(name="stg", bufs=2, space="PSUM") as stg,
            tc.tile_pool(name="acc", bufs=2, space="PSUM") as accp,
            tc.tile_pool(name="util", bufs=2, space="PSUM") as util,
        ):
            # ---------- constants / weights ----------
            # startup-critical order: wk[hp0] -> xT(tb0) -> wq[hp0] -> xoT ->
            # wv -> rest (interleaved below with the xT loads)
            wq_sb = singles.tile([P, G2, 2, E], FP8, name="wq_sb")
            wk_sb = singles.tile([P, G2, 2, E], FP8, name="wk_sb")
            wv_sb = singles.tile([P, G2, 2, E], FP8, name="wv_sb")
            nc.sync.dma_start(wk_sb[:, :, :, 0:P], wk[:, :, :, 0:P])
            # wo in per-head layout: [64, H, E]
            wo_sb = singles.tile([D, H, E], F16, name="wo_sb")
            # per-e_out-column biases for k^T/q^T ([128, 4] with col = block)
            bk_sb = singles.tile([P, EC], F32, name="bk_sb")
            bq_sb = singles.tile([P, EC], F32, name="bq_sb")
            nc.gpsimd.dma_start(bk_sb, bk.rearrange("(c p) -> p c", p=P))
            nc.gpsimd.dma_start(bq_sb, bq.rearrange("(c p) -> p c", p=P))
            # free-dim broadcast tiles
            bv_bc = singles.tile([P, E], F32, name="bv_bc")
            bo_bc = singles.tile([P, E], F32, name="bo_bc")
            g_bc = singles.tile([P, E], F32, name="g_bc")
            b_bc = singles.tile([P, E], F32, name="b_bc")
            for dst, src in ((bv_bc, bv), (bo_bc, bo), (g_bc, ln_g), (b_bc, ln_b)):
                nc.gpsimd.dma_start(out=dst, in_=src[None, :].to_broadcast((P, E)))
            eps_t = singles.tile([P, 1], F32, name="eps_t")
            nc.vector.memset(eps_t, 1e-5)
            shift_t = singles.tile([P, 1], F32, name="shift_t")
            nc.vector.memset(shift_t, EXP_SHIFT)
            ones16 = singles.tile([1, D], F16, name="ones16")
            nc.vector.memset(ones16, 1.0)
            # for folding bo into the O-proj as a K=1 matmul
            ones_p = singles.tile([1, P], F16, name="ones_p")
            nc.vector.memset(ones_p, 1.0)
            bo16 = singles.tile([1, E], F16, name="bo16")
            nc.scalar.activation(out=bo16, in_=bo_bc[0:1, :], func=AF.Copy)

            # ---------- x^T loads (pre-transposed fp8 on host) ----------
            xT = singles.tile([P, G2, 2, seq], FP8, name="xT")
            xoT = singles.tile([P, G2, 2, rows], FP8, name="xoT")
            def _w_rest_0():
                nc.gpsimd.dma_start(wq_sb[:, :, :, 0:P], wq[:, :, :, 0:P])
                for g in range(G2):
                    eng = nc.sync if g % 2 == 0 else nc.gpsimd
                    eng.dma_start(xoT[:, g, :, :], xoT_d[:, g, :, :])
                nc.sync.dma_start(wv_sb, wv)

            def _w_rest_1():
                nc.gpsimd.dma_start(wk_sb[:, :, :, P:E], wk[:, :, :, P:E])
                nc.sync.dma_start(wq_sb[:, :, :, P:E], wq[:, :, :, P:E])

            def _w_rest_2():
                nc.gpsimd.dma_start(wo_sb, wo)

            w_rest = [_w_rest_0, _w_rest_1, _w_rest_2]
            for tb in range(TB):
                for g in range(G2):
                    eng = nc.sync if g % 2 == 0 else nc.gpsimd
                    eng.dma_start(
                        xT[:, g, :, ds(tb * kblk, kblk)],
                        xT_d[:, g, :, ds(tb * kblk, kblk)],
                    )
                if w_rest:
                    w_rest.pop(0)()
            while w_rest:
                w_rest.pop(0)()

            # ---------- V projection (+bias, +ones col) ----------
            # fp8 DoubleRow pair layout: v2[pair][p, i, h, 0:64] = v[t=2*pair+i],
            # [..., 64] = 1.0 (softmax denominator column), rest zero-padding.
            v_tiles = {}
            def emit_v(t):
                pair, i = divmod(t, 2)
                if i == 0:
                    vt = vtiles.tile([P, 2, H, VP], FP8, name=f"v_{pair}", tag="v")
                    nc.vector.memset(vt[:, :, :, D:VP], 0.0)
                    nc.vector.memset(vt[:, :, :, D : D + 1], 1.0)
                    v_tiles[pair] = vt
                vt = v_tiles[pair]
                ps = util.tile([P, E], F32, name="v_ps", tag="u")
                for g in range(G2):
                    nc.tensor.matmul(
                        ps, lhsT=xT[:, g, :, ts(t, P)], rhs=wv_sb[:, g, :, :],
                        start=(g == 0), stop=(g == G2 - 1),
                        perf_mode=mybir.MatmulPerfMode.DoubleRow,
                    )
                nc.vector.tensor_add(
                    out=vt[:, i, :, 0:D],
                    in0=ps.rearrange("p (h d) -> p h d", h=H),
                    in1=bv_bc.rearrange("p (h d) -> p h d", h=H),
                )

            # ---------- K^T / Q^T projections (per head-pair block) ----------
            kT = [kqv.tile([P, seq], F16, name=f"kT_{hp}") for hp in range(HPAIRS)]
            qT = [kqv.tile([P, rows], F16, name=f"qT_{hp}") for hp in range(HPAIRS)]

            def emit_k(hp, tb):
                ps = util.tile([P, 512], F32, name="k_ps", tag="u")
                for g in range(G2):
                    nc.tensor.matmul(
                        ps[:, :kblk], lhsT=wk_sb[:, g, :, ds(hp * P, P)],
                        rhs=xT[:, g, :, ds(tb * kblk, kblk)],
                        start=(g == 0), stop=(g == G2 - 1),
                        perf_mode=mybir.MatmulPerfMode.DoubleRow,
                    )
                nc.vector.tensor_tensor(
                    kT[hp][:, ds(tb * kblk, kblk)], ps[:, :kblk],
                    bk_sb[:, hp : hp + 1].to_broadcast((P, kblk)),
                    mybir.AluOpType.add,
                )

            def emit_q(hp, rb):
                ps = util.tile([P, 512], F32, name="q_ps", tag="u")
                for g in range(G2):
                    nc.tensor.matmul(
                        ps[:, :qblk], lhsT=wq_sb[:, g, :, ds(hp * P, P)],
                        rhs=xoT[:, g, :, ds(rb * qblk, qblk)],
                        start=(g == 0), stop=(g == G2 - 1),
                        perf_mode=mybir.MatmulPerfMode.DoubleRow,
                    )
                nc.vector.tensor_tensor(
                    qT[hp][:, ds(rb * qblk, qblk)], ps[:, :qblk],
                    bq_sb[:, hp : hp + 1].to_broadcast((P, qblk)),
                    mybir.AluOpType.add,
                )

            # ---------- attention ----------
            scale = 1.0 / np.sqrt(D)
            exp_ctr = [0]
            # deferred normalize closures: each block's [recip -> PE
            # broadcast -> ctxT multiply] tail is emitted a few groups into
            # the NEXT block's attention, so the in-order PE queue never
            # stalls on the drain/reciprocal chain at block boundaries.
            pending_norm = []
            # ctx^T accumulators (f32, SBUF), one per head in the pair
            def attention(hp, sb, fillers, dve_exp=True, norm_first=False):
                if norm_first and pending_norm:
                    pending_norm.pop(0)()
                ctx_ps = [
                    accp.tile([VP, sblk], F32, name=f"ctx_{h}", tag="ctx")
                    for h in range(2)
                ]
                n_groups = (TT + exp_group - 1) // exp_group
                pending = []  # (at_tile, t0, gsz) awaiting A@V, one group behind
                def flush_av(last):
                    at_p, t0p, gszp = pending.pop(0)
                    assert gszp == 2, "DoubleRow A@V needs t-tile pairs"
                    for h in range(2):
                        nc.tensor.matmul(
                            ctx_ps[h][:, :sblk],
                            lhsT=v_tiles[t0p // 2][:, :, hp * 2 + h, :],
                            rhs=at_p[h][:, :, :sblk],
                            start=(t0p == 0), stop=last,
                            perf_mode=mybir.MatmulPerfMode.DoubleRow,
                        )
                for g in range(n_groups):
                    if g == 2 and not norm_first and pending_norm:
                        pending_norm.pop(0)()
                    t0 = g * exp_group
                    gsz = min(exp_group, TT - t0)
                    at_pair = []
                    for h in range(2):
                        st_t = stg.tile([P, exp_group, 512], F32, name=f"stg_{h}", tag="stg")
                        # scores (2 heads packed via PE row groups)
                        for j in range(gsz):
                            nc.tensor.matmul(
                                st_t[:, j, :sblk],
                                lhsT=kT[hp][ds(h * D, D), ts(t0 + j, P)],
                                rhs=qT[hp][ds(h * D, D), ds(sb * sblk, sblk)],
                                start=True, stop=True,
                                tile_position=(h * D, 0),
                            )
                        # exp (with 1/sqrt(D) folded in), PSUM -> SBUF fp8.
                        # Split across Scalar (true exp) and DVE (log-domain
                        # bit trick) to break the Scalar-engine bottleneck.
                        at_t = atp.tile([P, exp_group, 512], FP8, name=f"at_{h}", tag="at")
                        if (dve_exp and exp_dve_mod > 0
                                and (exp_ctr[0] * 2) % exp_dve_mod < 2):
                            nc.vector.tensor_scalar(
                                out=at_t[:, :gsz, :sblk].bitcast(mybir.dt.uint8),
                                in0=st_t[:, :gsz, :sblk],
                                scalar1=BITS_MUL, scalar2=BITS_ADD,
                                op0=mybir.AluOpType.mult,
                                op1=mybir.AluOpType.add,
                            )
                        else:
                            nc.scalar.activation(
                                out=at_t[:, :gsz, :sblk], in_=st_t[:, :gsz, :sblk],
                                func=AF.Exp, scale=scale, bias=shift_t,
                            )
                        exp_ctr[0] += 1
                        at_pair.append(at_t)
                    pending.append((at_pair, t0, gsz))
                    if len(pending) > 1:
                        flush_av(False)
                    if fillers:
                        fillers.pop(0)()
                flush_av(True)
                # drain on Scalar (frees DVE + PSUM acc banks); the denom row
                # hops to partition 0 by DMA for the custom-DVE reciprocal
                ctx_sb = [ctxp.tile([D + 1, sblk], F32, name=f"cs_{h}", tag="cs")
                          for h in range(2)]
                den0 = [normp.tile([1, sblk], F32, name=f"den0_{h}", tag="dn")
                        for h in range(2)]
                for h in range(2):
                    nc.scalar.activation(
                        out=ctx_sb[h], in_=ctx_ps[h][: D + 1, :sblk],
                        func=AF.Copy,
                    )
                    nc.sync.dma_start(den0[h], ctx_sb[h][D : D + 1, :])

                def do_norm(hp=hp, sb=sb, ctx_sb=ctx_sb, den0=den0):
                    for h in range(2):
                        recip = normp.tile([1, sblk], F32, name="recip")
                        nc.vector.reciprocal_approx_fast(
                            out=recip, in_=den0[h]
                        )
                        recip16 = normp.tile([1, sblk], F16, name="recip16")
                        nc.scalar.activation(
                            out=recip16, in_=recip, func=AF.Copy
                        )
                        rb_ps = util.tile([D, 512], F32, name="rb_ps", tag="u")
                        nc.tensor.matmul(
                            rb_ps[:, :sblk], lhsT=ones16, rhs=recip16,
                            start=True, stop=True,
                        )
                        nc.vector.tensor_mul(
                            out=ctxT[hp * 2 + h][:, ds(sb * sblk, sblk)],
                            in0=ctx_sb[h][0:D, :], in1=rb_ps[:, :sblk],
                        )

                pending_norm.append(do_norm)

            ctxT = [kqv.tile([D, rows], F16, name=f"ctxT_{h}") for h in range(H)]

            # ---------- O-projection + residual + LayerNorm ----------
            def emit_out(st):
                ps = util.tile([P, E], F32, name="o_ps", tag="u")
                for h in range(H):
                    nc.tensor.matmul(
                        ps, lhsT=ctxT[h][:, ts(st, P)], rhs=wo_sb[:, h, :],
                        start=(h == 0), stop=False,
                    )
                # bo folded in as a K=1 ones x bo outer product
                nc.tensor.matmul(
                    ps, lhsT=ones_p, rhs=bo16, start=False, stop=True
                )
                xo_t = youtp.tile([P, E], F32, name="xo_t")
                nc.sync.dma_start(xo_t, xo_f32[ts(st, P), :])
                y_t = youtp.tile([P, E], F32, name="y_t")
                nc.vector.tensor_add(out=y_t, in0=ps, in1=xo_t)
                # LayerNorm
                stats = normp.tile([P, 6], F32, name="stats")
                nc.vector.bn_stats(out=stats, in_=y_t)
                mv = normp.tile([P, 2], F32, name="mv")
                nc.vector.bn_aggr(out=mv, in_=stats)
                std = normp.tile([P, 1], F32, name="std")
                nc.scalar.activation(
                    out=std, in_=mv[:, 1:2], func=AF.Sqrt, bias=eps_t
                )
                nc.vector.reciprocal(out=std, in_=std)
                # (y - mu) * rstd fused in one pass (per-partition scalars)
                nc.vector.tensor_scalar(
                    out=y_t, in0=y_t, scalar1=mv[:, 0:1], scalar2=std,
                    op0=mybir.AluOpType.subtract, op1=mybir.AluOpType.mult,
                )
                nc.vector.tensor_mul(out=y_t, in0=y_t, in1=g_bc)
                nc.vector.tensor_add(out=y_t, in0=y_t, in1=b_bc)
                nc.sync.dma_start(y_out[ts(st, P), :], y_t)


            # ---------- emission order ----------
            # K(hp0)/Q(hp0) first so attention starts ASAP; V pairs 0-3 up
            # front, the rest streamed as fillers inside hp0's attention
            # (one pair per group, staying >=4 pairs ahead of the A@V flush).
            emitted_out = set()
            tpb = kblk // P  # t-tiles per k-block
            for tb in range(TB):
                emit_k(0, tb)
                if tb == 0:
                    for rb in range(QB):
                        emit_q(0, rb)
                for t in range(tb * tpb, (tb + 1) * tpb):
                    emit_v(t)

            for hp in range(HPAIRS):
                fillers = []
                if hp + 1 < HPAIRS:
                    nhp = hp + 1
                    for tb in range(TB):
                        fillers.append(lambda nhp=nhp, tb=tb: emit_k(nhp, tb))
                    for rb in range(QB):
                        fillers.append(lambda nhp=nhp, rb=rb: emit_q(nhp, rb))
                elif SB > 1:
                    # last head-pair: stream first s-block's output tiles
                    def of(st):
                        def run():
                            emit_out(st)
                            emitted_out.add(st)
                        return run
                per_sb = (len(fillers) + 1) // SB if fillers else 0
                for sb in range(SB):
                    last_att = hp == HPAIRS - 1 and sb == SB - 1 and SB > 1
                    if last_att:
                        chunk = [of(st) for st in range(ST // SB)]
                    else:
                        chunk = fillers[:per_sb]
                        del fillers[:per_sb]
                    # the streamed emit_outs of the last block read ctxT of
                    # the previous block -> its normalize must run up front
                    attention(hp, sb, chunk, norm_first=last_att)
                    for f in chunk:
                        f()
                for f in fillers:
                    f()

            while pending_norm:
                pending_norm.pop(0)()
            for st in range(ST):
                if st not in emitted_out:
                    emit_out(st)

    return nc


_CACHED = {}


def _get_nc(seq=S, rows=R, exp_group=2, sblk=512, exp_dve_mod=EXP_DVE_MOD):
    key = (seq, rows, exp_group, sblk, exp_dve_mod)
    if key not in _CACHED:
        nc = bacc.Bacc("TRN2", target_bir_lowering=False, debug=False,
                       num_devices=N_CORES)
        build_mha(nc, seq=seq, rows=rows, exp_group=exp_group, sblk=sblk,
                  exp_dve_mod=exp_dve_mod)
        nc.compile()
        _CACHED[key] = nc
    return _CACHED[key]


def pack_fp8_dr(x2d):
    """[S, E] f32 -> [128, G2, 2, S] fp8 with out[p, g, i, t] = x[t, (2g+i)*128+p]."""
    f8 = ml_dtypes.float8_e4m3
    s, e = x2d.shape
    return np.ascontiguousarray(
        x2d.T.reshape(e // 256, 2, 128, s).transpose(2, 0, 1, 3).astype(f8)
    )


def packw_fp8_dr(w):
    """[E, E] f32 -> [128, G2, 2, E] fp8 with out[p, g, i, e] = w[(2g+i)*128+p, e]."""
    f8 = ml_dtypes.float8_e4m3
    e_in, e_out = w.shape
    return np.ascontiguousarray(
        np.asarray(w, np.float32)
        .reshape(e_in // 256, 2, 128, e_out)
        .transpose(2, 0, 1, 3)
        .astype(f8)
    )


def packw_o(w):
    """[E, E] f32 -> [64, H, E] f16 with out[p, h, e] = w[h*64+p, e]."""
    w = np.asarray(w, np.float16)
    return np.ascontiguousarray(w.reshape(H, 64, E).transpose(1, 0, 2))


def make_in_maps(inputs):
    """Shard full inputs into per-core input dicts."""
    x = np.asarray(inputs["x"], np.float32)
    shared = {
        "wq_f8": packw_fp8_dr(inputs["wq"]),
        "wk_f8": packw_fp8_dr(inputs["wk"]),
        "wv_f8": packw_fp8_dr(inputs["wv"]),
        "wo_f16": packw_o(inputs["wo"]),
        "bq": np.asarray(inputs["bq"], np.float32),
        "bk": np.asarray(inputs["bk"], np.float32),
        "bv": np.asarray(inputs["bv"], np.float32),
        "bo": np.asarray(inputs["bo"], np.float32),
        "ln_g": np.asarray(inputs["ln_g"], np.float32),
        "ln_b": np.asarray(inputs["ln_b"], np.float32),
    }
    xT_all = [pack_fp8_dr(x[b]) for b in range(B)]
    in_maps = []
    for c in range(N_CORES):
        b, shard = divmod(c, SEQ_SHARDS)
        r0 = shard * R
        m = dict(shared)
        m["xT_f8"] = xT_all[b]
        m["xoT_f8"] = np.ascontiguousarray(xT_all[b][:, :, :, r0 : r0 + R])
        m["xo_f32"] = np.ascontiguousarray(x[b, r0 : r0 + R])
        in_maps.append(m)
    return in_maps


def kernel(**inputs):
    from concourse import bass_utils

    nc = _get_nc()
    in_maps = make_in_maps(inputs)
    res = bass_utils.run_bass_kernel_spmd(nc, in_maps, core_ids=list(range(N_CORES)))
    out = np.empty((B, S, E), np.float32)
    for c in range(N_CORES):
        b, shard = divmod(c, SEQ_SHARDS)
        out[b, shard * R : (shard + 1) * R] = res.results[c]["y"]
    return out



# revision 83
# speedup vs baseline: 1.1355x; 1.0003x over previous
"""Trainium2 Bass kernel for a full MHA block (QKV proj + softmax attention +
output proj + residual + LayerNorm), B=2, S=4096, E=512, H=8, D=64.

Sharding: sequence-parallel over 8 cores (4 seq shards x 2 batches). Each core
owns R=1024 query rows of one batch, recomputes K/V for the full context
(avoids all cross-core communication), and writes its own [R, E] output slice.

Layout strategy (per core):
  - x^T via XBAR DMA-transpose (bf16) -> [128, 4, S]
  - K^T, Q^T projections in head-major layout [e_out(=2 heads)/128, t]
  - scores computed transposed: S_T[t, s] = K^T.T @ Q^T, two heads packed into
    PE row groups (K=64 each), PSUM out [t=128, s=512]
  - exp on ScalarE straight from PSUM (scale=1/8 free), out bf16 A_T tiles
  - A@V: lhsT = [V_h | ones] (65 cols) so row 64 of the product accumulates the
    softmax denominator for free; accumulated per 3-chunk group in PSUM then
    drained into an SBUF f32 accumulator by VectorE (keeps PSUM pressure at 8
    banks: 2x3 score staging + 2 utility)
  - normalize: reciprocal of denom row, partition-broadcast via SBUF->SBUF DMA,
    VectorE multiply -> ctx^T bf16 (per-head tiles, base partition 0 always)
  - O-proj: per-head K=64 matmuls accumulating all 8 heads, + residual (f32) +
    LayerNorm (bn_stats/bn_aggr) on VectorE/ScalarE, f32 output.
"""

import sys

sys.path.insert(0, "/opt/trn_rl_repo")

import numpy as np
import ml_dtypes

import concourse.bass as bass
import concourse.bacc as bacc
import concourse.mybir as mybir
import concourse.tile as tile
from concourse.bass import ds, ts

# Problem constants (hardcoded per harness contract)
B = 2
S = 4096
E = 512
H = 8
D = 64
N_CORES = 8
SEQ_SHARDS = N_CORES // B
R = S // SEQ_SHARDS  # 1024 own query rows per core

F32 = mybir.dt.float32
F32R = mybir.dt.float32r
F16 = mybir.dt.float16
FP8 = mybir.dt.float8e4
VP = 80  # padded V columns (64 V + 1 ones + pad to a 16-multiple for DoubleRow)
EXP_SHIFT = -3.0  # exp(s/8 - 3): keeps exp outputs < fp8e4 max; cancels in softmax
# log-domain exp on DVE: fp8e4m3 bits of exp(s/8+SHIFT) == s*BITS_MUL + BITS_ADD,
# rounded + saturated to [0,255] by the uint8 convert (verified on HW)
BITS_MUL = 11.5416529 / 8.0
BITS_ADD = 56.0 + 11.5416529 * EXP_SHIFT
EXP_DVE_MOD = 8  # 2 of every 8 exp tiles go to DVE (bit-trick), rest to Scalar
AF = mybir.ActivationFunctionType


def build_mha(nc, seq=S, rows=R, exp_group=2, sblk=512, exp_dve_mod=EXP_DVE_MOD):
    """Emit the Tile program. seq/rows shrinkable for simulation."""
    P = 128
    EC = E // P           # 4 e_in chunks
    HPAIRS = H // 2       # 4 head-pair blocks (=e_out blocks of 128)
    TT = seq // P         # t tiles
    kblk = min(512, seq)
    TB = seq // kblk      # t blocks for K-proj
    qblk = min(512, rows)
    QB = rows // qblk     # r blocks for Q-proj
    sblk = min(sblk, rows)
    SB = rows // sblk     # s blocks per core
    ST = rows // P        # s tiles for O-proj/LN

    G2 = E // 256         # DoubleRow chunk-pair groups for the projections

    # ---- DRAM I/O ----
    # x fed pre-transposed+packed fp8 from host:
    #   xT[p, g, i, t] = x[t, (2g+i)*128+p]   (DoubleRow pair axis i)
    xT_d = nc.dram_tensor("xT_f8", [P, G2, 2, seq], FP8, kind="ExternalInput").ap()
    xoT_d = nc.dram_tensor("xoT_f8", [P, G2, 2, rows], FP8, kind="ExternalInput").ap()
    xo_f32 = nc.dram_tensor("xo_f32", [rows, E], F32, kind="ExternalInput").ap()
    # weights pre-packed on host: wX[p, g, i, e] = w[(2g+i)*128+p, e];
    # wo[p, h, e] = wo[h*64+p, e]
    wq = nc.dram_tensor("wq_f8", [P, G2, 2, E], FP8, kind="ExternalInput").ap()
    wk = nc.dram_tensor("wk_f8", [P, G2, 2, E], FP8, kind="ExternalInput").ap()
    wv = nc.dram_tensor("wv_f8", [P, G2, 2, E], FP8, kind="ExternalInput").ap()
    wo = nc.dram_tensor("wo_f16", [D, H, E], F16, kind="ExternalInput").ap()
    bq = nc.dram_tensor("bq", [E], F32, kind="ExternalInput").ap()
    bk = nc.dram_tensor("bk", [E], F32, kind="ExternalInput").ap()
    bv = nc.dram_tensor("bv", [E], F32, kind="ExternalInput").ap()
    bo = nc.dram_tensor("bo", [E], F32, kind="ExternalInput").ap()
    ln_g = nc.dram_tensor("ln_g", [E], F32, kind="ExternalInput").ap()
    ln_b = nc.dram_tensor("ln_b", [E], F32, kind="ExternalInput").ap()
    y_out = nc.dram_tensor("y", [rows, E], F32, kind="ExternalOutput").ap()

    with tile.TileContext(nc) as tc:
        with (
            tc.tile_pool(name="singles", bufs=1) as singles,
            tc.tile_pool(name="kqv", bufs=1) as kqv,
            tc.tile_pool(name="vtiles", bufs=TT) as vtiles,
            tc.tile_pool(name="at", bufs=4) as atp,
            tc.tile_pool(name="ctx", bufs=4) as ctxp,
            tc.tile_pool(name="norm", bufs=4) as normp,
            tc.tile_pool(name="yout", bufs=3) as youtp,
            tc.tile_pool(name="stg", bufs=2, space="PSUM") as stg,
            tc.tile_pool(name="acc", bufs=2, space="PSUM") as accp,
            tc.tile_pool(name="util", bufs=2, space="PSUM") as util,
        ):
            # ---------- constants / weights ----------
            # startup-critical order: wk[hp0] -> xT(tb0) -> wq[hp0] -> xoT ->
            # wv -> rest (interleaved below with the xT loads)
            wq_sb = singles.tile([P, G2, 2, E], FP8, name="wq_sb")
            wk_sb = singles.tile([P, G2, 2, E], FP8, name="wk_sb")
            wv_sb = singles.tile([P, G2, 2, E], FP8, name="wv_sb")
            nc.sync.dma_start(wk_sb[:, :, :, 0:P], wk[:, :, :, 0:P])
            # wo in per-head layout: [64, H, E]
            wo_sb = singles.tile([D, H, E], F16, name="wo_sb")
            # per-e_out-column biases for k^T/q^T ([128, 4] with col = block)
            bk_sb = singles.tile([P, EC], F32, name="bk_sb")
            bq_sb = singles.tile([P, EC], F32, name="bq_sb")
            nc.gpsimd.dma_start(bk_sb, bk.rearrange("(c p) -> p c", p=P))
            nc.gpsimd.dma_start(bq_sb, bq.rearrange("(c p) -> p c", p=P))
            # free-dim broadcast tiles
            bv_bc = singles.tile([P, E], F32, name="bv_bc")
            bo_bc = singles.tile([P, E], F32, name="bo_bc")
            g_bc = singles.tile([P, E], F32, name="g_bc")
            b_bc = singles.tile([P, E], F32, name="b_bc")
            for dst, src in ((bv_bc, bv), (bo_bc, bo), (g_bc, ln_g), (b_bc, ln_b)):
                nc.gpsimd.dma_start(out=dst, in_=src[None, :].to_broadcast((P, E)))
            eps_t = singles.tile([P, 1], F32, name="eps_t")
            nc.vector.memset(eps_t, 1e-5)
            shift_t = singles.tile([P, 1], F32, name="shift_t")
            nc.vector.memset(shift_t, EXP_SHIFT)
            ones16 = singles.tile([1, D], F16, name="ones16")
            nc.vector.memset(ones16, 1.0)
            # for folding bo into the O-proj as a K=1 matmul
            ones_p = singles.tile([1, P], F16, name="ones_p")
            nc.vector.memset(ones_p, 1.0)
            bo16 = singles.tile([1, E], F16, name="bo16")
            nc.scalar.activation(out=bo16, in_=bo_bc[0:1, :], func=AF.Copy)

            # ---------- x^T loads (pre-transposed fp8 on host) ----------
            xT = singles.tile([P, G2, 2, seq], FP8, name="xT")
            xoT = singles.tile([P, G2, 2, rows], FP8, name="xoT")
            def _w_rest_0():
                nc.gpsimd.dma_start(wq_sb[:, :, :, 0:P], wq[:, :, :, 0:P])
                for g in range(G2):
                    eng = nc.sync if g % 2 == 0 else nc.gpsimd
                    eng.dma_start(xoT[:, g, :, :], xoT_d[:, g, :, :])
                nc.sync.dma_start(wv_sb, wv)

            def _w_rest_1():
                nc.gpsimd.dma_start(wk_sb[:, :, :, P:E], wk[:, :, :, P:E])
                nc.sync.dma_start(wq_sb[:, :, :, P:E], wq[:, :, :, P:E])

            def _w_rest_2():
                nc.gpsimd.dma_start(wo_sb, wo)

            w_rest = [_w_rest_0, _w_rest_1, _w_rest_2]
            for tb in range(TB):
                for g in range(G2):
                    eng = nc.sync if g % 2 == 0 else nc.gpsimd
                    eng.dma_start(
                        xT[:, g, :, ds(tb * kblk, kblk)],
                        xT_d[:, g, :, ds(tb * kblk, kblk)],
                    )
                if w_rest:
                    w_rest.pop(0)()
            while w_rest:
                w_rest.pop(0)()

            # ---------- V projection (+bias, +ones col) ----------
            # fp8 DoubleRow pair layout: v2[pair][p, i, h, 0:64] = v[t=2*pair+i],
            # [..., 64] = 1.0 (softmax denominator column), rest zero-padding.
            v_tiles = {}
            def emit_v(t):
                pair, i = divmod(t, 2)
                if i == 0:
                    vt = vtiles.tile([P, 2, H, VP], FP8, name=f"v_{pair}", tag="v")
                    nc.vector.memset(vt[:, :, :, D:VP], 0.0)
                    nc.vector.memset(vt[:, :, :, D : D + 1], 1.0)
                    v_tiles[pair] = vt
                vt = v_tiles[pair]
                ps = util.tile([P, E], F32, name="v_ps", tag="u")
                for g in range(G2):
                    nc.tensor.matmul(
                        ps, lhsT=xT[:, g, :, ts(t, P)], rhs=wv_sb[:, g, :, :],
                        start=(g == 0), stop=(g == G2 - 1),
                        perf_mode=mybir.MatmulPerfMode.DoubleRow,
                    )
                nc.vector.tensor_add(
                    out=vt[:, i, :, 0:D],
                    in0=ps.rearrange("p (h d) -> p h d", h=H),
                    in1=bv_bc.rearrange("p (h d) -> p h d", h=H),
                )

            # ---------- K^T / Q^T projections (per head-pair block) ----------
            kT = [kqv.tile([P, seq], F16, name=f"kT_{hp}") for hp in range(HPAIRS)]
            qT = [kqv.tile([P, rows], F16, name=f"qT_{hp}") for hp in range(HPAIRS)]

            def emit_k(hp, tb):
                ps = util.tile([P, 512], F32, name="k_ps", tag="u")
                for g in range(G2):
                    nc.tensor.matmul(
                        ps[:, :kblk], lhsT=wk_sb[:, g, :, ds(hp * P, P)],
                        rhs=xT[:, g, :, ds(tb * kblk, kblk)],
                        start=(g == 0), stop=(g == G2 - 1),
                        perf_mode=mybir.MatmulPerfMode.DoubleRow,
                    )
                nc.vector.tensor_tensor(
                    kT[hp][:, ds(tb * kblk, kblk)], ps[:, :kblk],
                    bk_sb[:, hp : hp + 1].to_broadcast((P, kblk)),
                    mybir.AluOpType.add,
                )

            def emit_q(hp, rb):
                ps = util.tile([P, 512], F32, name="q_ps", tag="u")
                for g in range(G2):
                    nc.tensor.matmul(
                        ps[:, :qblk], lhsT=wq_sb[:, g, :, ds(hp * P, P)],
                        rhs=xoT[:, g, :, ds(rb * qblk, qblk)],
                        start=(g == 0), stop=(g == G2 - 1),
                        perf_mode=mybir.MatmulPerfMode.DoubleRow,
                    )
                nc.vector.tensor_tensor(
                    qT[hp][:, ds(rb * qblk, qblk)], ps[:, :qblk],
                    bq_sb[:, hp : hp + 1].to_broadcast((P, qblk)),
                    mybir.AluOpType.add,
                )

            # ---------- attention ----------
            scale = 1.0 / np.sqrt(D)
            exp_ctr = [0]
            # deferred normalize closures: each block's [recip -> PE
            # broadcast -> ctxT multiply] tail is emitted a few groups into
            # the NEXT block's attention, so the in-order PE queue never
            # stalls on the drain/reciprocal chain at block boundaries.
            pending_norm = []
            # ctx^T accumulators (f32, SBUF), one per head in the pair
            def attention(hp, sb, fillers, dve_exp=True, norm_first=False):
                if norm_first and pending_norm:
                    pending_norm.pop(0)()
                ctx_ps = [
                    accp.tile([VP, sblk], F32, name=f"ctx_{h}", tag="ctx")
                    for h in range(2)
                ]
                n_groups = (TT + exp_group - 1) // exp_group
                pending = []  # (at_tile, t0, gsz) awaiting A@V, one group behind
                def flush_av(last):
                    at_p, t0p, gszp = pending.pop(0)
                    assert gszp == 2, "DoubleRow A@V needs t-tile pairs"
                    for h in range(2):
                        nc.tensor.matmul(
                            ctx_ps[h][:, :sblk],
                            lhsT=v_tiles[t0p // 2][:, :, hp * 2 + h, :],
                            rhs=at_p[h][:, :, :sblk],
                            start=(t0p == 0), stop=last,
                            perf_mode=mybir.MatmulPerfMode.DoubleRow,
                        )
                for g in range(n_groups):
                    if g == 2 and not norm_first and pending_norm:
                        pending_norm.pop(0)()
                    t0 = g * exp_group
                    gsz = min(exp_group, TT - t0)
                    at_pair = []
                    for h in range(2):
                        st_t = stg.tile([P, exp_group, 512], F32, name=f"stg_{h}", tag="stg")
                        # scores (2 heads packed via PE row groups)
                        for j in range(gsz):
                            nc.tensor.matmul(
                                st_t[:, j, :sblk],
                                lhsT=kT[hp][ds(h * D, D), ts(t0 + j, P)],
                                rhs=qT[hp][ds(h * D, D), ds(sb * sblk, sblk)],
                                start=True, stop=True,
                                tile_position=(h * D, 0),
                            )
                        # exp (with 1/sqrt(D) folded in), PSUM -> SBUF fp8.
                        # Split across Scalar (true exp) and DVE (log-domain
                        # bit trick) to break the Scalar-engine bottleneck.
                        at_t = atp.tile([P, exp_group, 512], FP8, name=f"at_{h}", tag="at")
                        if (dve_exp and exp_dve_mod > 0
                                and (exp_ctr[0] * 2) % exp_dve_mod < 2):
                            nc.vector.tensor_scalar(
                                out=at_t[:, :gsz, :sblk].bitcast(mybir.dt.uint8),
                                in0=st_t[:, :gsz, :sblk],
                                scalar1=BITS_MUL, scalar2=BITS_ADD,
                                op0=mybir.AluOpType.mult,
                                op1=mybir.AluOpType.add,
                            )
                        else:
                            nc.scalar.activation(
                                out=at_t[:, :gsz, :sblk], in_=st_t[:, :gsz, :sblk],
                                func=AF.Exp, scale=scale, bias=shift_t,
                            )
                        exp_ctr[0] += 1
                        at_pair.append(at_t)
                    pending.append((at_pair, t0, gsz))
                    if len(pending) > 1:
                        flush_av(False)
                    if fillers:
                        fillers.pop(0)()
                flush_av(True)
                # drain on Scalar (frees DVE + PSUM acc banks); the denom row
                # hops to partition 0 by DMA for the custom-DVE reciprocal
                ctx_sb = [ctxp.tile([D + 1, sblk], F32, name=f"cs_{h}", tag="cs")
                          for h in range(2)]
                den0 = [normp.tile([1, sblk], F32, name=f"den0_{h}", tag="dn")
                        for h in range(2)]
                for h in range(2):
                    nc.vector.tensor_copy(ctx_sb[h], ctx_ps[h][: D + 1, :sblk])
                    nc.sync.dma_start(den0[h], ctx_sb[h][D : D + 1, :])

                def do_norm(hp=hp, sb=sb, ctx_sb=ctx_sb, den0=den0):
                    for h in range(2):
                        recip = normp.tile([1, sblk], F32, name="recip")
                        nc.vector.reciprocal_approx_fast(
                            out=recip, in_=den0[h]
                        )
                        recip16 = normp.tile([1, sblk], F16, name="recip16")
                        nc.scalar.activation(
                            out=recip16, in_=recip, func=AF.Copy
                        )
                        rb_ps = util.tile([D, 512], F32, name="rb_ps", tag="u")
                        nc.tensor.matmul(
                            rb_ps[:, :sblk], lhsT=ones16, rhs=recip16,
                            start=True, stop=True,
                        )
                        nc.vector.tensor_mul(
                            out=ctxT[hp * 2 + h][:, ds(sb * sblk, sblk)],
                            in0=ctx_sb[h][0:D, :], in1=rb_ps[:, :sblk],
                        )

                pending_norm.append(do_norm)

            ctxT = [kqv.tile([D, rows], F16, name=f"ctxT_{h}") for h in range(H)]

            # ---------- O-projection + residual + LayerNorm ----------
            def emit_out(st):
                ps = util.tile([P, E], F32, name="o_ps", tag="u")
                for h in range(H):
                    nc.tensor.matmul(
                        ps, lhsT=ctxT[h][:, ts(st, P)], rhs=wo_sb[:, h, :],
                        start=(h == 0), stop=False,
                    )
                # bo folded in as a K=1 ones x bo outer product
                nc.tensor.matmul(
                    ps, lhsT=ones_p, rhs=bo16, start=False, stop=True
                )
                xo_t = youtp.tile([P, E], F32, name="xo_t")
                nc.sync.dma_start(xo_t, xo_f32[ts(st, P), :])
                y_t = youtp.tile([P, E], F32, name="y_t")
                nc.vector.tensor_add(out=y_t, in0=ps, in1=xo_t)
                # LayerNorm
                stats = normp.tile([P, 6], F32, name="stats")
                nc.vector.bn_stats(out=stats, in_=y_t)
                mv = normp.tile([P, 2], F32, name="mv")
                nc.vector.bn_aggr(out=mv, in_=stats)
                std = normp.tile([P, 1], F32, name="std")
                nc.scalar.activation(
                    out=std, in_=mv[:, 1:2], func=AF.Sqrt, bias=eps_t
                )
                nc.vector.reciprocal(out=std, in_=std)
                # (y - mu) * rstd fused in one pass (per-partition scalars)
                nc.vector.tensor_scalar(
                    out=y_t, in0=y_t, scalar1=mv[:, 0:1], scalar2=std,
                    op0=mybir.AluOpType.subtract, op1=mybir.AluOpType.mult,
                )
                nc.vector.tensor_mul(out=y_t, in0=y_t, in1=g_bc)
                nc.vector.tensor_add(out=y_t, in0=y_t, in1=b_bc)
                nc.sync.dma_start(y_out[ts(st, P), :], y_t)


            # ---------- emission order ----------
            # K(hp0)/Q(hp0) first so attention starts ASAP; V pairs 0-3 up
            # front, the rest streamed as fillers inside hp0's attention
            # (one pair per group, staying >=4 pairs ahead of the A@V flush).
            emitted_out = set()
            tpb = kblk // P  # t-tiles per k-block
            for tb in range(TB):
                emit_k(0, tb)
                if tb == 0:
                    for rb in range(QB):
                        emit_q(0, rb)
                for t in range(tb * tpb, (tb + 1) * tpb):
                    emit_v(t)

            for hp in range(HPAIRS):
                fillers = []
                if hp + 1 < HPAIRS:
                    nhp = hp + 1
                    for tb in range(TB):
                        fillers.append(lambda nhp=nhp, tb=tb: emit_k(nhp, tb))
                    for rb in range(QB):
                        fillers.append(lambda nhp=nhp, rb=rb: emit_q(nhp, rb))
                elif SB > 1:
                    # last head-pair: stream first s-block's output tiles
                    def of(st):
                        def run():
                            emit_out(st)
                            emitted_out.add(st)
                        return run
                per_sb = (len(fillers) + 1) // SB if fillers else 0
                for sb in range(SB):
                    last_att = hp == HPAIRS - 1 and sb == SB - 1 and SB > 1
                    if last_att:
                        chunk = [of(st) for st in range(ST // SB)]
                    else:
                        chunk = fillers[:per_sb]
                        del fillers[:per_sb]
                    # the streamed emit_outs of the last block read ctxT of
                    # the previous block -> its normalize must run up front
                    attention(hp, sb, chunk, norm_first=last_att)
                    for f in chunk:
                        f()
                for f in fillers:
                    f()

            while pending_norm:
                pending_norm.pop(0)()
            for st in range(ST):
                if st not in emitted_out:
                    emit_out(st)

    return nc


_CACHED = {}


def _get_nc(seq=S, rows=R, exp_group=2, sblk=512, exp_dve_mod=EXP_DVE_MOD):
    key = (seq, rows, exp_group, sblk, exp_dve_mod)
    if key not in _CACHED:
        nc = bacc.Bacc("TRN2", target_bir_lowering=False, debug=False,
                       num_devices=N_CORES)
        build_mha(nc, seq=seq, rows=rows, exp_group=exp_group, sblk=sblk,
                  exp_dve_mod=exp_dve_mod)
        nc.compile()
        _CACHED[key] = nc
    return _CACHED[key]


def pack_fp8_dr(x2d):
    """[S, E] f32 -> [128, G2, 2, S] fp8 with out[p, g, i, t] = x[t, (2g+i)*128+p]."""
    f8 = ml_dtypes.float8_e4m3
    s, e = x2d.shape
    return np.ascontiguousarray(
        x2d.T.reshape(e // 256, 2, 128, s).transpose(2, 0, 1, 3).astype(f8)
    )


def packw_fp8_dr(w):
    """[E, E] f32 -> [128, G2, 2, E] fp8 with out[p, g, i, e] = w[(2g+i)*128+p, e]."""
    f8 = ml_dtypes.float8_e4m3
    e_in, e_out = w.shape
    return np.ascontiguousarray(
        np.asarray(w, np.float32)
        .reshape(e_in // 256, 2, 128, e_out)
        .transpose(2, 0, 1, 3)
        .astype(f8)
    )


def packw_o(w):
    """[E, E] f32 -> [64, H, E] f16 with out[p, h, e] = w[h*64+p, e]."""
    w = np.asarray(w, np.float16)
    return np.ascontiguousarray(w.reshape(H, 64, E).transpose(1, 0, 2))


def make_in_maps(inputs):
    """Shard full inputs into per-core input dicts."""
    x = np.asarray(inputs["x"], np.float32)
    shared = {
        "wq_f8": packw_fp8_dr(inputs["wq"]),
        "wk_f8": packw_fp8_dr(inputs["wk"]),
        "wv_f8": packw_fp8_dr(inputs["wv"]),
        "wo_f16": packw_o(inputs["wo"]),
        "bq": np.asarray(inputs["bq"], np.float32),
        "bk": np.asarray(inputs["bk"], np.float32),
        "bv": np.asarray(inputs["bv"], np.float32),
        "bo": np.asarray(inputs["bo"], np.float32),
        "ln_g": np.asarray(inputs["ln_g"], np.float32),
        "ln_b": np.asarray(inputs["ln_b"], np.float32),
    }
    xT_all = [pack_fp8_dr(x[b]) for b in range(B)]
    in_maps = []
    for c in range(N_CORES):
        b, shard = divmod(c, SEQ_SHARDS)
        r0 = shard * R
        m = dict(shared)
        m["xT_f8"] = xT_all[b]
        m["xoT_f8"] = np.ascontiguousarray(xT_all[b][:, :, :, r0 : r0 + R])
        m["xo_f32"] = np.ascontiguousarray(x[b, r0 : r0 + R])
        in_maps.append(m)
    return in_maps


def kernel(**inputs):
    from concourse import bass_utils

    nc = _get_nc()
    in_maps = make_in_maps(inputs)
    res = bass_utils.run_bass_kernel_spmd(nc, in_maps, core_ids=list(range(N_CORES)))
    out = np.empty((B, S, E), np.float32)
    for c in range(N_CORES):
        b, shard = divmod(c, SEQ_SHARDS)
        out[b, shard * R : (shard + 1) * R] = res.results[c]["y"]
    return out

